# revision 19
# baseline (speedup 1.0000x reference)
"""Trainium2 Bass kernel for FFTResonanceBlock.

Math (per flattened resonator b of B=256, freq bin c of 1025, frame t of 128):
  coeffs = 0.5 + sigmoid(decay)*0.5*0.99
  mags[t]  = amp^2 * coeffs^(t+1)              (geometric scan on DVE)
  phase[t] = (t+1)*tanh(phase)*pi + tanh(dith)*Sn[t]   (Sn = const noise cumsum)
  spec = mags * exp(i*phase); frames = irfft(spec, 2048); overlap-add hop 1024.

Device strategy (8 cores, 32 resonators each, embarrassingly parallel):
  - tiles (c_chunk=128 partitions, t=128 free) per resonator
  - sin/cos via magic-number round + cody-waite reduction + Sin LUT
  - irfft + overlap-add as ONE halved matmul using
      D[c, 1024+r] = (-1)^c D[c, r]:
    out[t,r] = sum_c (re[c,t] + (-1)^c re[c,t-1]) C[c,r]
             + (im[c,t] + (-1)^c im[c,t-1]) S[c,r]
    with the sigma-combine done on GpSimd, products split DVE/GpSimd.
"""
import sys

sys.path.insert(0, "/opt/trn_rl_repo")

import numpy as np
import ml_dtypes

import concourse.bass as bass  # noqa: F401
import concourse.mybir as mybir
import concourse.tile as tile
from concourse import bacc, bass_utils

# ---- problem constants (hardcoded per spec) ----
N_CORES = 8
N_RES, EXPR = 64, 4
B = N_RES * EXPR          # 256 flattened resonators
BL = B // N_CORES         # 32 per core
C = 1025                  # rfft bins for window 2048
CP = 1152                 # padded to 9*128
NCH = CP // 128           # 9 c-chunks
T = 128                   # frames
W = 2048                  # window
HOP = 1024
N_SAMPLES = T * HOP       # 131072
BASE_RES = 0.5
RES_FACTOR = 0.99

PI = float(np.pi)
TWO_PI = 2.0 * np.pi
INV2PI = float(1.0 / TWO_PI)
MAGIC = float(np.float32(1.5 * 2**23))
CW1 = float(np.float32(6.28125))
CW2 = float(np.float32(TWO_PI - 6.28125))
CW3 = float(np.float32(TWO_PI - 6.28125 - float(np.float32(TWO_PI - 6.28125))))

F32 = mybir.dt.float32
BF16 = mybir.dt.bfloat16

_CACHE: dict = {}


def _constants():
    """Input-independent constants: noise cumsum, halved DFT mats, t-vec, sign."""
    if "consts" in _CACHE:
        return _CACHE["consts"]
    import jax

    cpu = jax.devices("cpu")[0]
    with jax.default_device(cpu):
        noise = jax.random.uniform(
            jax.random.key(42), (B, T, C), minval=-1.0, maxval=1.0
        )
        noise = np.asarray(noise, dtype=np.float32)
    sn = np.cumsum(noise, axis=1, dtype=np.float32)        # (B, T, C)
    sn_p = np.zeros((B, CP, T), dtype=np.float32)
    sn_p[:, :C, :] = np.transpose(sn, (0, 2, 1))
    # descriptor-friendly layout: (B, p, ch, t) so each partition is one
    # contiguous 4.6KB run
    sn_h = np.ascontiguousarray(
        sn_p.reshape(B, NCH, 128, T).transpose(0, 2, 1, 3)
    ).reshape(B, 128, NCH * T)

    # halved DFT matrices, cols 0..1023 (second half folded via (-1)^c)
    k = np.arange(CP, dtype=np.float64)[:, None]
    n = np.arange(HOP, dtype=np.float64)[None, :]
    ang = 2.0 * np.pi * k * n / W
    w = np.full((CP, 1), 2.0 / W)
    w[0, 0] = 1.0 / W
    w[C - 1, 0] = 1.0 / W
    w[C:, 0] = 0.0
    cm = (w * np.cos(ang)).astype(ml_dtypes.bfloat16)              # (1152, 1024)
    smat = (-(w * np.sin(ang))[:1024]).astype(ml_dtypes.bfloat16)  # (1024, 1024)

    tb = np.broadcast_to(
        np.arange(1, T + 1, dtype=np.float32)[None, :], (128, T)
    ).copy()
    sgn = np.where(np.arange(128) % 2 == 0, 1.0, -1.0).astype(np.float32)
    sgn = sgn.reshape(128, 1)

    _CACHE["consts"] = (sn_h, cm, smat, tb, sgn)
    return _CACHE["consts"]


def _build_program():
    if "nc" in _CACHE:
        return _CACHE["nc"]

    nc = bacc.Bacc("TRN2", target_bir_lowering=False, debug=False, num_devices=1)

    par_d = nc.dram_tensor("par", (CP, 4 * BL), F32, kind="ExternalInput").ap()
    spb_d = nc.dram_tensor("spB", (128, NCH * BL), F32, kind="ExternalInput").ap()
    ddb_d = nc.dram_tensor("ddB", (128, NCH * BL), F32, kind="ExternalInput").ap()
    sn_d = nc.dram_tensor("snt", (BL, 128, NCH * T), F32, kind="ExternalInput").ap()
    tb_d = nc.dram_tensor("tb", (128, T), F32, kind="ExternalInput").ap()
    sg_d = nc.dram_tensor("sgn", (128, 1), F32, kind="ExternalInput").ap()
    cm_d = nc.dram_tensor("cmat", (CP, HOP), BF16, kind="ExternalInput").ap()
    sm_d = nc.dram_tensor("smat", (1024, HOP), BF16, kind="ExternalInput").ap()
    out_d = nc.dram_tensor("out", (BL, N_SAMPLES), F32, kind="ExternalOutput").ap()

    with tile.TileContext(nc) as tc:
        with (
            tc.tile_pool(name="const", bufs=1) as cpool,
            tc.tile_pool(name="sn", bufs=3) as snpool,
            tc.tile_pool(name="tf", bufs=12) as tfpool,
            tc.tile_pool(name="th", bufs=5) as thpool,
            tc.tile_pool(name="mg", bufs=10) as mgpool,
            tc.tile_pool(name="spec", bufs=2) as spool,
            tc.tile_pool(name="ola", bufs=3) as opool,
            tc.tile_pool(name="ps", bufs=4, space="PSUM") as ppool,
        ):
            # small constants first (unblocks the b=0 compute chain)
            par_t = []
            for ch in range(NCH):
                t_ = cpool.tile([128, 4 * BL], F32, tag=f"par{ch}")
                nc.sync.dma_start(out=t_[:], in_=par_d[ch * 128:(ch + 1) * 128, :])
                par_t.append(t_)
            tb_t = cpool.tile([128, T], F32, tag="tb")
            nc.sync.dma_start(out=tb_t[:], in_=tb_d[:])
            sg_t = cpool.tile([128, 1], F32, tag="sgn")
            nc.sync.dma_start(out=sg_t[:], in_=sg_d[:])
            spb_t = cpool.tile([128, NCH * BL], F32, tag="spB")
            nc.sync.dma_start(out=spb_t[:], in_=spb_d[:])
            ddb_t = cpool.tile([128, NCH * BL], F32, tag="ddB")
            nc.sync.dma_start(out=ddb_t[:], in_=ddb_d[:])
            # DFT matrices on the SWDGE queue, chunk 0 first
            cm_t, sm_t = [], []
            for ch in range(NCH):
                ct = cpool.tile([128, HOP], BF16, tag=f"cm{ch}")
                nc.gpsimd.dma_start(out=ct[:], in_=cm_d[ch * 128:(ch + 1) * 128, :])
                cm_t.append(ct)
                if ch < NCH - 1:
                    st = cpool.tile([128, HOP], BF16, tag=f"sm{ch}")
                    nc.gpsimd.dma_start(
                        out=st[:], in_=sm_d[ch * 128:(ch + 1) * 128, :]
                    )
                    sm_t.append(st)

            FD = NCH * T      # 1152
            FDP = NCH * (T + 1)  # 1161, padded spec layout
            GRP = 8           # resonators per exp/sin table-load phase

            for g in range(BL // GRP):
              mags_of = {}
              # ---- phase A: decay envelopes (Exp table set) ----
              # mags = exp(lc*(t+1) + lnsm), fused into the activation's
              # per-partition scale/bias — no DVE involvement.
              for b in range(g * GRP, (g + 1) * GRP):
                mg = mgpool.tile([128, FD], BF16, tag="mg")
                for ch in range(NCH):
                    sl = slice(ch * T, (ch + 1) * T)
                    nc.scalar.activation(
                        mg[:, sl], tb_t[:], mybir.ActivationFunctionType.Exp,
                        bias=par_t[ch][:, 3 * BL + b:3 * BL + b + 1],
                        scale=par_t[ch][:, 2 * BL + b:2 * BL + b + 1],
                    )
                mags_of[b] = mg
              # ---- phase B: phase accumulation + sin/cos + DFT (Sin set) ----
              for b in range(g * GRP, (g + 1) * GRP):
                sn_t_ = snpool.tile([128, FD], F32, tag="sn")
                nc.sync.dma_start(out=sn_t_[:], in_=sn_d[b])

                spc = lambda ch: par_t[ch][:, b:b + 1]                 # noqa: E731
                ddc = lambda ch: par_t[ch][:, BL + b:BL + b + 1]       # noqa: E731
                mags = mags_of[b]

                # p1[p, ch*T+t] = sp[ch*128+p] * (t+1): one GpSimd op with
                # tb broadcast over chunks and spB broadcast over t
                p1 = tfpool.tile([128, FD], F32, tag="tf")
                v3f = lambda tl: tl[:].rearrange(                     # noqa: E731
                    "p (c t) -> p c t", c=NCH)
                tb_b = tb_t[:].rearrange("p (o t) -> p o t", o=1).to_broadcast(
                    (128, NCH, T))
                spb_b = spb_t[:].rearrange(
                    "p (c bb) -> p c bb", bb=BL)[:, :, b:b + 1].to_broadcast(
                    (128, NCH, T))
                nc.gpsimd.tensor_tensor(
                    v3f(p1), tb_b, spb_b, mybir.AluOpType.mult
                )
                # acc = dd*Sn + p1: two full-width DVE ops
                ddb_b = ddb_t[:].rearrange(
                    "p (c bb) -> p c bb", bb=BL)[:, :, b:b + 1].to_broadcast(
                    (128, NCH, T))
                snd = tfpool.tile([128, FD], F32, tag="tf")
                nc.vector.tensor_tensor(
                    v3f(snd), v3f(sn_t_), ddb_b, mybir.AluOpType.mult
                )
                acc = tfpool.tile([128, FD], F32, tag="tf")
                nc.vector.tensor_tensor(
                    acc[:], snd[:], p1[:], mybir.AluOpType.add
                )

                t1 = tfpool.tile([128, FD], F32, tag="tf")
                nc.vector.tensor_scalar(
                    t1[:], acc[:], INV2PI, MAGIC,
                    mybir.AluOpType.mult, mybir.AluOpType.add,
                )
                kk = tfpool.tile([128, FD], F32, tag="tf")
                nc.vector.tensor_scalar(
                    kk[:], t1[:], MAGIC, None, mybir.AluOpType.subtract
                )
                red = tfpool.tile([128, FD], F32, tag="tf")
                nc.vector.cody_waite_cascade(red[:], acc[:], kk[:], CW1, CW2, CW3)
                redc = tfpool.tile([128, FD], F32, tag="tf")
                nc.vector.add_range_wrap(redc[:], red[:], PI / 2, PI, TWO_PI)

                sinv = thpool.tile([128, FD], BF16, tag="th")
                nc.scalar.activation(sinv[:], red[:], mybir.ActivationFunctionType.Sin)
                cosv = thpool.tile([128, FD], BF16, tag="th")
                nc.scalar.activation(cosv[:], redc[:], mybir.ActivationFunctionType.Sin)

                # spectra in padded (129/chunk) layout, zero pad col for t-shift
                re_t = spool.tile([128, FDP], BF16, tag="re")
                im_t = spool.tile([128, FDP], BF16, tag="im")
                re3 = re_t[:].rearrange("p (c t) -> p c t", c=NCH)
                im3 = im_t[:].rearrange("p (c t) -> p c t", c=NCH)
                nc.gpsimd.memset(re3[:, :, 0:1], 0.0)
                nc.gpsimd.memset(im3[:, :, 0:1], 0.0)
                v3 = lambda tl: tl[:].rearrange("p (c t) -> p c t", c=NCH)  # noqa: E731
                nc.gpsimd.tensor_tensor(
                    re3[:, :, 1:], v3(mags), v3(cosv), mybir.AluOpType.mult
                )
                nc.gpsimd.tensor_tensor(
                    im3[:, :, 1:], v3(mags), v3(sinv), mybir.AluOpType.mult
                )

                # sigma-combine: rea[c,t] = sgn*re[c,t-1] + re[c,t]
                # (scalar_tensor_tensor is DVE-only; Pool rejects it)
                rea = spool.tile([128, FD], BF16, tag="rea")
                ima = spool.tile([128, FD], BF16, tag="ima")
                nc.vector.scalar_tensor_tensor(
                    v3(rea), re3[:, :, 0:T], sg_t[:], re3[:, :, 1:T + 1],
                    mybir.AluOpType.mult, mybir.AluOpType.add,
                )
                nc.vector.scalar_tensor_tensor(
                    v3(ima), im3[:, :, 0:T], sg_t[:], im3[:, :, 1:T + 1],
                    mybir.AluOpType.mult, mybir.AluOpType.add,
                )

                ps = ppool.tile([128, HOP], F32, tag="ps")
                pairs = []
                for ch in range(NCH):
                    pairs.append((rea, cm_t[ch], ch))
                    if ch < NCH - 1:
                        pairs.append((ima, sm_t[ch], ch))
                n_mm = len(pairs)
                for j in range(2):
                    for idx, (spec_t, mat_t, ch) in enumerate(pairs):
                        nc.tensor.matmul(
                            ps[:, j * 512:(j + 1) * 512],
                            spec_t[:, ch * T:(ch + 1) * T],
                            mat_t[:, j * 512:(j + 1) * 512],
                            start=(idx == 0), stop=(idx == n_mm - 1),
                        )

                ola = opool.tile([128, HOP], F32, tag="ola")
                nc.scalar.copy(ola[:], ps[:])
                nc.sync.dma_start(
                    out=out_d[b].rearrange("(t r) -> t r", t=T), in_=ola[:]
                )

    nc.compile()
    _CACHE["nc"] = nc
    return nc


def _prep_inputs(amp, phase, decay, phase_dither):
    """Host prep: flatten, derive per-(b,c) scalars, pad, pack to (CP, 4B)."""

    def flat(x):
        return np.transpose(np.asarray(x, np.float32), (0, 2, 1)).reshape(B, C)

    amp_f, phase_f, decay_f, dith_f = map(flat, (amp, phase, decay, phase_dither))
    gg = (BASE_RES + (1.0 / (1.0 + np.exp(-decay_f))) * (1.0 - BASE_RES) * RES_FACTOR)
    lc = np.log(gg)
    lnsm = 2.0 * np.log(np.maximum(amp_f, 1e-30))
    sp = np.tanh(phase_f) * np.pi
    dd = np.tanh(dith_f)

    def padT(x, padval):
        o = np.full((CP, B), padval, dtype=np.float32)
        o[:C, :] = x.T.astype(np.float32)
        return o

    return padT(sp, 1.0), padT(dd, 0.5), padT(lc, -0.2), padT(lnsm, -1.4)


def kernel(amp, phase, decay, phase_dither):
    sn_h, cm, smat, tb, sgn = _constants()
    sp, dd, gg, sm = _prep_inputs(amp, phase, decay, phase_dither)
    nc = _build_program()

    def chunk_major(x):  # (CP, BL) -> (128, NCH*BL): [p, ch*BL+b]
        return np.ascontiguousarray(
            x.reshape(NCH, 128, BL).transpose(1, 0, 2).reshape(128, NCH * BL)
        )

    in_maps = []
    for core in range(N_CORES):
        bs = slice(core * BL, (core + 1) * BL)
        par = np.concatenate([sp[:, bs], dd[:, bs], gg[:, bs], sm[:, bs]], axis=1)
        in_maps.append({
            "par": np.ascontiguousarray(par),
            "spB": chunk_major(sp[:, bs]),
            "ddB": chunk_major(dd[:, bs]),
            "snt": sn_h[bs],
            "tb": tb,
            "sgn": sgn,
            "cmat": cm,
            "smat": smat,
        })

    res = bass_utils.run_bass_kernel_spmd(
        nc, in_maps, core_ids=list(range(N_CORES))
    )
    out = np.concatenate([r["out"] for r in res.results], axis=0)  # (256, 131072)
    return out.reshape(1, 1, N_RES, EXPR, N_SAMPLES)


# revision 22
# speedup vs baseline: 1.2177x; 1.2177x over previous
"""Trainium2 Bass kernel for FFTResonanceBlock.

Math (per flattened resonator b of B=256, freq bin c of 1025, frame t of 128):
  coeffs = 0.5 + sigmoid(decay)*0.5*0.99
  mags[t]  = amp^2 * coeffs^(t+1)              (geometric scan on DVE)
  phase[t] = (t+1)*tanh(phase)*pi + tanh(dith)*Sn[t]   (Sn = const noise cumsum)
  spec = mags * exp(i*phase); frames = irfft(spec, 2048); overlap-add hop 1024.

Device strategy (8 cores, 32 resonators each, embarrassingly parallel):
  - tiles (c_chunk=128 partitions, t=128 free) per resonator
  - sin/cos via magic-number round + cody-waite reduction + Sin LUT
  - irfft + overlap-add as ONE halved matmul using
      D[c, 1024+r] = (-1)^c D[c, r]:
    out[t,r] = sum_c (re[c,t] + (-1)^c re[c,t-1]) C[c,r]
             + (im[c,t] + (-1)^c im[c,t-1]) S[c,r]
    with the sigma-combine done on GpSimd, products split DVE/GpSimd.
"""
import sys

sys.path.insert(0, "/opt/trn_rl_repo")

import numpy as np
import ml_dtypes

import concourse.bass as bass  # noqa: F401
import concourse.mybir as mybir
import concourse.tile as tile
from concourse import bacc, bass_utils

# ---- problem constants (hardcoded per spec) ----
N_CORES = 8
N_RES, EXPR = 64, 4
B = N_RES * EXPR          # 256 flattened resonators
BL = B // N_CORES         # 32 per core
C = 1025                  # rfft bins for window 2048
CP = 1152                 # padded to 9*128
NCH = CP // 128           # 9 c-chunks
T = 128                   # frames
W = 2048                  # window
HOP = 1024
N_SAMPLES = T * HOP       # 131072
BASE_RES = 0.5
RES_FACTOR = 0.99

PI = float(np.pi)
TWO_PI = 2.0 * np.pi
INV2PI = float(1.0 / TWO_PI)
MAGIC = float(np.float32(1.5 * 2**23))
CW1 = float(np.float32(6.28125))
CW2 = float(np.float32(TWO_PI - 6.28125))
CW3 = float(np.float32(TWO_PI - 6.28125 - float(np.float32(TWO_PI - 6.28125))))

F32 = mybir.dt.float32
BF16 = mybir.dt.bfloat16

_CACHE: dict = {}


def _constants():
    """Input-independent constants: noise cumsum, halved DFT mats, t-vec, sign."""
    if "consts" in _CACHE:
        return _CACHE["consts"]
    import jax

    cpu = jax.devices("cpu")[0]
    with jax.default_device(cpu):
        noise = jax.random.uniform(
            jax.random.key(42), (B, T, C), minval=-1.0, maxval=1.0
        )
        noise = np.asarray(noise, dtype=np.float32)
    sn = np.cumsum(noise, axis=1, dtype=np.float32)        # (B, T, C)
    sn_p = np.zeros((B, CP, T), dtype=np.float32)
    sn_p[:, :C, :] = np.transpose(sn, (0, 2, 1))
    # descriptor-friendly layout: (B, p, ch, t) so each partition is one
    # contiguous 4.6KB run
    sn_h = np.ascontiguousarray(
        sn_p.reshape(B, NCH, 128, T).transpose(0, 2, 1, 3)
    ).reshape(B, 128, NCH * T)

    # halved DFT matrices, cols 0..1023 (second half folded via (-1)^c)
    k = np.arange(CP, dtype=np.float64)[:, None]
    n = np.arange(HOP, dtype=np.float64)[None, :]
    ang = 2.0 * np.pi * k * n / W
    w = np.full((CP, 1), 2.0 / W)
    w[0, 0] = 1.0 / W
    w[C - 1, 0] = 1.0 / W
    w[C:, 0] = 0.0
    cm = (w * np.cos(ang)).astype(ml_dtypes.bfloat16)              # (1152, 1024)
    smat = (-(w * np.sin(ang))[:1024]).astype(ml_dtypes.bfloat16)  # (1024, 1024)

    tb = np.broadcast_to(
        np.arange(1, T + 1, dtype=np.float32)[None, :], (128, T)
    ).copy()
    sgn = np.where(np.arange(128) % 2 == 0, 1.0, -1.0).astype(np.float32)
    sgn = sgn.reshape(128, 1)

    _CACHE["consts"] = (sn_h, cm, smat, tb, sgn)
    return _CACHE["consts"]


def _build_program():
    if "nc" in _CACHE:
        return _CACHE["nc"]

    nc = bacc.Bacc("TRN2", target_bir_lowering=False, debug=False, num_devices=1)

    par_d = nc.dram_tensor("par", (CP, 4 * BL), F32, kind="ExternalInput").ap()
    spb_d = nc.dram_tensor("spB", (128, NCH * BL), F32, kind="ExternalInput").ap()
    ddb_d = nc.dram_tensor("ddB", (128, NCH * BL), F32, kind="ExternalInput").ap()
    sn_d = nc.dram_tensor("snt", (BL, 128, NCH * T), F32, kind="ExternalInput").ap()
    tb_d = nc.dram_tensor("tb", (128, T), F32, kind="ExternalInput").ap()
    sg_d = nc.dram_tensor("sgn", (128, 1), F32, kind="ExternalInput").ap()
    cm_d = nc.dram_tensor("cmat", (CP, HOP), BF16, kind="ExternalInput").ap()
    sm_d = nc.dram_tensor("smat", (1024, HOP), BF16, kind="ExternalInput").ap()
    out_d = nc.dram_tensor("out", (BL, N_SAMPLES), F32, kind="ExternalOutput").ap()

    with tile.TileContext(nc) as tc:
        with (
            tc.tile_pool(name="const", bufs=1) as cpool,
            tc.tile_pool(name="sn", bufs=3) as snpool,
            tc.tile_pool(name="tf", bufs=12) as tfpool,
            tc.tile_pool(name="th", bufs=5) as thpool,
            tc.tile_pool(name="mg", bufs=10) as mgpool,
            tc.tile_pool(name="spec", bufs=2) as spool,
            tc.tile_pool(name="ola", bufs=3) as opool,
            tc.tile_pool(name="ps", bufs=4, space="PSUM") as ppool,
        ):
            # small constants first (unblocks the b=0 compute chain)
            par_t = []
            for ch in range(NCH):
                t_ = cpool.tile([128, 4 * BL], F32, tag=f"par{ch}")
                nc.sync.dma_start(out=t_[:], in_=par_d[ch * 128:(ch + 1) * 128, :])
                par_t.append(t_)
            tb_t = cpool.tile([128, T], F32, tag="tb")
            nc.sync.dma_start(out=tb_t[:], in_=tb_d[:])
            sg_t = cpool.tile([128, 1], F32, tag="sgn")
            nc.sync.dma_start(out=sg_t[:], in_=sg_d[:])
            spb_t = cpool.tile([128, NCH * BL], F32, tag="spB")
            nc.sync.dma_start(out=spb_t[:], in_=spb_d[:])
            ddb_t = cpool.tile([128, NCH * BL], F32, tag="ddB")
            nc.sync.dma_start(out=ddb_t[:], in_=ddb_d[:])
            # DFT matrices on the SWDGE queue, chunk 0 first
            cm_t, sm_t = [], []
            for ch in range(NCH):
                ct = cpool.tile([128, HOP], BF16, tag=f"cm{ch}")
                nc.gpsimd.dma_start(out=ct[:], in_=cm_d[ch * 128:(ch + 1) * 128, :])
                cm_t.append(ct)
                if ch < NCH - 1:
                    st = cpool.tile([128, HOP], BF16, tag=f"sm{ch}")
                    nc.gpsimd.dma_start(
                        out=st[:], in_=sm_d[ch * 128:(ch + 1) * 128, :]
                    )
                    sm_t.append(st)

            FD = NCH * T      # 1152
            FDP = NCH * (T + 1)  # 1161, padded spec layout
            GRP = 8           # resonators per exp/sin table-load phase

            for g in range(BL // GRP):
              mags_of = {}
              # ---- phase A: decay envelopes (Exp table set) ----
              # mags = exp(lc*(t+1) + lnsm), fused into the activation's
              # per-partition scale/bias — no DVE involvement.
              for b in range(g * GRP, (g + 1) * GRP):
                mg = mgpool.tile([128, FD], BF16, tag="mg")
                for ch in range(NCH):
                    sl = slice(ch * T, (ch + 1) * T)
                    nc.scalar.activation(
                        mg[:, sl], tb_t[:], mybir.ActivationFunctionType.Exp,
                        bias=par_t[ch][:, 3 * BL + b:3 * BL + b + 1],
                        scale=par_t[ch][:, 2 * BL + b:2 * BL + b + 1],
                    )
                mags_of[b] = mg
              # ---- phase B: phase accumulation + sin/cos + DFT (Sin set) ----
              for b in range(g * GRP, (g + 1) * GRP):
                sn_t_ = snpool.tile([128, FD], F32, tag="sn")
                nc.sync.dma_start(out=sn_t_[:], in_=sn_d[b])

                spc = lambda ch: par_t[ch][:, b:b + 1]                 # noqa: E731
                ddc = lambda ch: par_t[ch][:, BL + b:BL + b + 1]       # noqa: E731
                mags = mags_of[b]

                # p1[p, ch*T+t] = sp[ch*128+p] * (t+1): one GpSimd op with
                # tb broadcast over chunks and spB broadcast over t
                p1 = tfpool.tile([128, FD], F32, tag="tf")
                v3f = lambda tl: tl[:].rearrange(                     # noqa: E731
                    "p (c t) -> p c t", c=NCH)
                tb_b = tb_t[:].rearrange("p (o t) -> p o t", o=1).to_broadcast(
                    (128, NCH, T))
                spb_b = spb_t[:].rearrange(
                    "p (c bb) -> p c bb", bb=BL)[:, :, b:b + 1].to_broadcast(
                    (128, NCH, T))
                nc.vector.tensor_tensor(
                    v3f(p1), tb_b, spb_b, mybir.AluOpType.mult
                )
                # acc = dd*Sn + p1: two full-width DVE ops
                ddb_b = ddb_t[:].rearrange(
                    "p (c bb) -> p c bb", bb=BL)[:, :, b:b + 1].to_broadcast(
                    (128, NCH, T))
                snd = tfpool.tile([128, FD], F32, tag="tf")
                nc.vector.tensor_tensor(
                    v3f(snd), v3f(sn_t_), ddb_b, mybir.AluOpType.mult
                )
                acc = tfpool.tile([128, FD], F32, tag="tf")
                nc.vector.tensor_tensor(
                    acc[:], snd[:], p1[:], mybir.AluOpType.add
                )

                t1 = tfpool.tile([128, FD], F32, tag="tf")
                nc.vector.tensor_scalar(
                    t1[:], acc[:], INV2PI, MAGIC,
                    mybir.AluOpType.mult, mybir.AluOpType.add,
                )
                kk = tfpool.tile([128, FD], F32, tag="tf")
                nc.vector.tensor_scalar(
                    kk[:], t1[:], MAGIC, None, mybir.AluOpType.subtract
                )
                red = tfpool.tile([128, FD], F32, tag="tf")
                nc.vector.cody_waite_cascade(red[:], acc[:], kk[:], CW1, CW2, CW3)
                redc = tfpool.tile([128, FD], F32, tag="tf")
                nc.vector.add_range_wrap(redc[:], red[:], PI / 2, PI, TWO_PI)

                sinv = thpool.tile([128, FD], BF16, tag="th")
                nc.scalar.activation(sinv[:], red[:], mybir.ActivationFunctionType.Sin)
                cosv = thpool.tile([128, FD], BF16, tag="th")
                nc.scalar.activation(cosv[:], redc[:], mybir.ActivationFunctionType.Sin)

                re_t = spool.tile([128, FD], BF16, tag="re")
                im_t = spool.tile([128, FD], BF16, tag="im")
                # contiguous bf16 products (DVE 2x mode)
                nc.vector.tensor_tensor(
                    re_t[:], mags[:], cosv[:], mybir.AluOpType.mult
                )
                nc.vector.tensor_tensor(
                    im_t[:], mags[:], sinv[:], mybir.AluOpType.mult
                )

                # sigma-combine: rea[c,t] = sgn*re[c,t-1] + re[c,t].
                # One full-width shifted STT; the t=0 column of each chunk
                # picks up the previous chunk's t=127 value, so overwrite
                # those 9 columns with a strided copy afterwards (Pool).
                rea = spool.tile([128, FD], BF16, tag="rea")
                ima = spool.tile([128, FD], BF16, tag="ima")
                nc.vector.scalar_tensor_tensor(
                    rea[:, 1:], re_t[:, :FD - 1], sg_t[:], re_t[:, 1:],
                    mybir.AluOpType.mult, mybir.AluOpType.add,
                )
                nc.vector.scalar_tensor_tensor(
                    ima[:, 1:], im_t[:, :FD - 1], sg_t[:], im_t[:, 1:],
                    mybir.AluOpType.mult, mybir.AluOpType.add,
                )
                nc.gpsimd.tensor_copy(v3f(rea)[:, :, 0:1], v3f(re_t)[:, :, 0:1])
                nc.gpsimd.tensor_copy(v3f(ima)[:, :, 0:1], v3f(im_t)[:, :, 0:1])

                ps = ppool.tile([128, HOP], F32, tag="ps")
                pairs = []
                for ch in range(NCH):
                    pairs.append((rea, cm_t[ch], ch))
                    if ch < NCH - 1:
                        pairs.append((ima, sm_t[ch], ch))
                n_mm = len(pairs)
                for j in range(2):
                    for idx, (spec_t, mat_t, ch) in enumerate(pairs):
                        nc.tensor.matmul(
                            ps[:, j * 512:(j + 1) * 512],
                            spec_t[:, ch * T:(ch + 1) * T],
                            mat_t[:, j * 512:(j + 1) * 512],
                            start=(idx == 0), stop=(idx == n_mm - 1),
                        )

                ola = opool.tile([128, HOP], F32, tag="ola")
                nc.scalar.copy(ola[:], ps[:])
                nc.sync.dma_start(
                    out=out_d[b].rearrange("(t r) -> t r", t=T), in_=ola[:]
                )

    nc.compile()
    _CACHE["nc"] = nc
    return nc


def _prep_inputs(amp, phase, decay, phase_dither):
    """Host prep: flatten, derive per-(b,c) scalars, pad, pack to (CP, 4B)."""

    def flat(x):
        return np.transpose(np.asarray(x, np.float32), (0, 2, 1)).reshape(B, C)

    amp_f, phase_f, decay_f, dith_f = map(flat, (amp, phase, decay, phase_dither))
    gg = (BASE_RES + (1.0 / (1.0 + np.exp(-decay_f))) * (1.0 - BASE_RES) * RES_FACTOR)
    lc = np.log(gg)
    lnsm = 2.0 * np.log(np.maximum(amp_f, 1e-30))
    sp = np.tanh(phase_f) * np.pi
    dd = np.tanh(dith_f)

    def padT(x, padval):
        o = np.full((CP, B), padval, dtype=np.float32)
        o[:C, :] = x.T.astype(np.float32)
        return o

    return padT(sp, 1.0), padT(dd, 0.5), padT(lc, -0.2), padT(lnsm, -1.4)


def kernel(amp, phase, decay, phase_dither):
    sn_h, cm, smat, tb, sgn = _constants()
    sp, dd, gg, sm = _prep_inputs(amp, phase, decay, phase_dither)
    nc = _build_program()

    def chunk_major(x):  # (CP, BL) -> (128, NCH*BL): [p, ch*BL+b]
        return np.ascontiguousarray(
            x.reshape(NCH, 128, BL).transpose(1, 0, 2).reshape(128, NCH * BL)
        )

    in_maps = []
    for core in range(N_CORES):
        bs = slice(core * BL, (core + 1) * BL)
        par = np.concatenate([sp[:, bs], dd[:, bs], gg[:, bs], sm[:, bs]], axis=1)
        in_maps.append({
            "par": np.ascontiguousarray(par),
            "spB": chunk_major(sp[:, bs]),
            "ddB": chunk_major(dd[:, bs]),
            "snt": sn_h[bs],
            "tb": tb,
            "sgn": sgn,
            "cmat": cm,
            "smat": smat,
        })

    res = bass_utils.run_bass_kernel_spmd(
        nc, in_maps, core_ids=list(range(N_CORES))
    )
    out = np.concatenate([r["out"] for r in res.results], axis=0)  # (256, 131072)
    return out.reshape(1, 1, N_RES, EXPR, N_SAMPLES)


# revision 24
# speedup vs baseline: 1.3105x; 1.0762x over previous
"""Trainium2 Bass kernel for FFTResonanceBlock.

Math (per flattened resonator b of B=256, freq bin c of 1025, frame t of 128):
  coeffs = 0.5 + sigmoid(decay)*0.5*0.99
  mags[t]  = amp^2 * coeffs^(t+1)              (geometric scan on DVE)
  phase[t] = (t+1)*tanh(phase)*pi + tanh(dith)*Sn[t]   (Sn = const noise cumsum)
  spec = mags * exp(i*phase); frames = irfft(spec, 2048); overlap-add hop 1024.

Device strategy (8 cores, 32 resonators each, embarrassingly parallel):
  - tiles (c_chunk=128 partitions, t=128 free) per resonator
  - sin/cos via magic-number round + cody-waite reduction + Sin LUT
  - irfft + overlap-add as ONE halved matmul using
      D[c, 1024+r] = (-1)^c D[c, r]:
    out[t,r] = sum_c (re[c,t] + (-1)^c re[c,t-1]) C[c,r]
             + (im[c,t] + (-1)^c im[c,t-1]) S[c,r]
    with the sigma-combine done on GpSimd, products split DVE/GpSimd.
"""
import sys

sys.path.insert(0, "/opt/trn_rl_repo")

import numpy as np
import ml_dtypes

import concourse.bass as bass  # noqa: F401
import concourse.mybir as mybir
import concourse.tile as tile
from concourse import bacc, bass_utils

# ---- problem constants (hardcoded per spec) ----
N_CORES = 8
N_RES, EXPR = 64, 4
B = N_RES * EXPR          # 256 flattened resonators
BL = B // N_CORES         # 32 per core
C = 1025                  # rfft bins for window 2048
CP = 1152                 # padded to 9*128
NCH = CP // 128           # 9 c-chunks
T = 128                   # frames
W = 2048                  # window
HOP = 1024
N_SAMPLES = T * HOP       # 131072
BASE_RES = 0.5
RES_FACTOR = 0.99

PI = float(np.pi)
TWO_PI = 2.0 * np.pi
INV2PI = float(1.0 / TWO_PI)
MAGIC = float(np.float32(1.5 * 2**23))
CW1 = float(np.float32(6.28125))
CW2 = float(np.float32(TWO_PI - 6.28125))
CW3 = float(np.float32(TWO_PI - 6.28125 - float(np.float32(TWO_PI - 6.28125))))

F32 = mybir.dt.float32
BF16 = mybir.dt.bfloat16
TWO_PI_F32 = float(np.float32(TWO_PI))

_CACHE: dict = {}


def _register_range_reduce_op():
    """Custom DVE op: red = x - round(x/2pi)*fl(2pi) in ONE instruction
    (magic-number round + single-constant reduction, ~2.7e-5 rad error)."""
    from concourse import dve_ops as dvo
    from concourse.dve_spec import Spec, Src0, C0, C1, C2

    name = "RANGE_REDUCE_2PI_ANT"
    if name in dvo._SUB_OPCODE_FOR_NAME:
        return next(o for o in dvo.OPS if o.name == name)
    k = (Src0 * C0 + C1) - C1
    op = dvo.DveOp(
        name,
        Spec(
            body=Src0 - k * C2,
            reference=lambda in0, s0, s1, imm2: (
                in0.astype(np.float32)
                - ((in0.astype(np.float32) * np.float32(s0) + np.float32(s1))
                   - np.float32(s1)) * np.float32(imm2)
            ),
        ),
        subdim=False,
        uops_sha={"v3": "7f89aefee880b3b9", "v4": "7f89aefee880b3b9"},
    )
    dvo.OPS.append(op)
    dvo._SUB_OPCODE_FOR_NAME[name] = max(dvo._SUB_OPCODE_FOR_NAME.values()) + 1
    dvo.CUSTOM_DVE_SPECS[name] = op.spec
    return op


_RR_OP = _register_range_reduce_op()


def _constants():
    """Input-independent constants: noise cumsum, halved DFT mats, t-vec, sign."""
    if "consts" in _CACHE:
        return _CACHE["consts"]
    import jax

    cpu = jax.devices("cpu")[0]
    with jax.default_device(cpu):
        noise = jax.random.uniform(
            jax.random.key(42), (B, T, C), minval=-1.0, maxval=1.0
        )
        noise = np.asarray(noise, dtype=np.float32)
    sn = np.cumsum(noise, axis=1, dtype=np.float32)        # (B, T, C)
    sn_p = np.zeros((B, CP, T), dtype=np.float32)
    sn_p[:, :C, :] = np.transpose(sn, (0, 2, 1))
    # descriptor-friendly layout: (B, p, ch, t) so each partition is one
    # contiguous 4.6KB run
    sn_h = np.ascontiguousarray(
        sn_p.reshape(B, NCH, 128, T).transpose(0, 2, 1, 3)
    ).reshape(B, 128, NCH * T)

    # halved DFT matrices, cols 0..1023 (second half folded via (-1)^c)
    k = np.arange(CP, dtype=np.float64)[:, None]
    n = np.arange(HOP, dtype=np.float64)[None, :]
    ang = 2.0 * np.pi * k * n / W
    w = np.full((CP, 1), 2.0 / W)
    w[0, 0] = 1.0 / W
    w[C - 1, 0] = 1.0 / W
    w[C:, 0] = 0.0
    cm = (w * np.cos(ang)).astype(ml_dtypes.bfloat16)              # (1152, 1024)
    smat = (-(w * np.sin(ang))[:1024]).astype(ml_dtypes.bfloat16)  # (1024, 1024)

    tb = np.broadcast_to(
        np.arange(1, T + 1, dtype=np.float32)[None, :], (128, T)
    ).copy()
    sgn = np.where(np.arange(128) % 2 == 0, 1.0, -1.0).astype(np.float32)
    sgn = sgn.reshape(128, 1)

    _CACHE["consts"] = (sn_h, cm, smat, tb, sgn)
    return _CACHE["consts"]


def _build_program():
    if "nc" in _CACHE:
        return _CACHE["nc"]

    nc = bacc.Bacc("TRN2", target_bir_lowering=False, debug=False, num_devices=1)

    par_d = nc.dram_tensor("par", (CP, 4 * BL), F32, kind="ExternalInput").ap()
    spb_d = nc.dram_tensor("spB", (128, NCH * BL), F32, kind="ExternalInput").ap()
    ddb_d = nc.dram_tensor("ddB", (128, NCH * BL), F32, kind="ExternalInput").ap()
    sn_d = nc.dram_tensor("snt", (BL, 128, NCH * T), F32, kind="ExternalInput").ap()
    tb_d = nc.dram_tensor("tb", (128, T), F32, kind="ExternalInput").ap()
    sg_d = nc.dram_tensor("sgn", (128, 1), F32, kind="ExternalInput").ap()
    cm_d = nc.dram_tensor("cmat", (CP, HOP), BF16, kind="ExternalInput").ap()
    sm_d = nc.dram_tensor("smat", (1024, HOP), BF16, kind="ExternalInput").ap()
    out_d = nc.dram_tensor("out", (BL, N_SAMPLES), F32, kind="ExternalOutput").ap()

    with tile.TileContext(nc) as tc:
        with (
            tc.tile_pool(name="const", bufs=1) as cpool,
            tc.tile_pool(name="sn", bufs=3) as snpool,
            tc.tile_pool(name="tf", bufs=12) as tfpool,
            tc.tile_pool(name="th", bufs=5) as thpool,
            tc.tile_pool(name="mg", bufs=10) as mgpool,
            tc.tile_pool(name="spec", bufs=2) as spool,
            tc.tile_pool(name="ola", bufs=3) as opool,
            tc.tile_pool(name="ps", bufs=4, space="PSUM") as ppool,
        ):
            # small constants first (unblocks the b=0 compute chain)
            par_t = []
            for ch in range(NCH):
                t_ = cpool.tile([128, 4 * BL], F32, tag=f"par{ch}")
                nc.sync.dma_start(out=t_[:], in_=par_d[ch * 128:(ch + 1) * 128, :])
                par_t.append(t_)
            tb_t = cpool.tile([128, T], F32, tag="tb")
            nc.sync.dma_start(out=tb_t[:], in_=tb_d[:])
            sg_t = cpool.tile([128, 1], F32, tag="sgn")
            nc.sync.dma_start(out=sg_t[:], in_=sg_d[:])
            spb_t = cpool.tile([128, NCH * BL], F32, tag="spB")
            nc.sync.dma_start(out=spb_t[:], in_=spb_d[:])
            ddb_t = cpool.tile([128, NCH * BL], F32, tag="ddB")
            nc.sync.dma_start(out=ddb_t[:], in_=ddb_d[:])
            # DFT matrices on the SWDGE queue, chunk 0 first
            cm_t, sm_t = [], []
            for ch in range(NCH):
                ct = cpool.tile([128, HOP], BF16, tag=f"cm{ch}")
                nc.gpsimd.dma_start(out=ct[:], in_=cm_d[ch * 128:(ch + 1) * 128, :])
                cm_t.append(ct)
                if ch < NCH - 1:
                    st = cpool.tile([128, HOP], BF16, tag=f"sm{ch}")
                    nc.gpsimd.dma_start(
                        out=st[:], in_=sm_d[ch * 128:(ch + 1) * 128, :]
                    )
                    sm_t.append(st)

            FD = NCH * T      # 1152
            FDP = NCH * (T + 1)  # 1161, padded spec layout
            GRP = 8           # resonators per exp/sin table-load phase

            for g in range(BL // GRP):
              mags_of = {}
              # ---- phase A: decay envelopes (Exp table set) ----
              # mags = exp(lc*(t+1) + lnsm), fused into the activation's
              # per-partition scale/bias — no DVE involvement.
              for b in range(g * GRP, (g + 1) * GRP):
                mg = mgpool.tile([128, FD], BF16, tag="mg")
                for ch in range(NCH):
                    sl = slice(ch * T, (ch + 1) * T)
                    nc.scalar.activation(
                        mg[:, sl], tb_t[:], mybir.ActivationFunctionType.Exp,
                        bias=par_t[ch][:, 3 * BL + b:3 * BL + b + 1],
                        scale=par_t[ch][:, 2 * BL + b:2 * BL + b + 1],
                    )
                mags_of[b] = mg
              # ---- phase B: phase accumulation + sin/cos + DFT (Sin set) ----
              for b in range(g * GRP, (g + 1) * GRP):
                sn_t_ = snpool.tile([128, FD], F32, tag="sn")
                nc.sync.dma_start(out=sn_t_[:], in_=sn_d[b])

                spc = lambda ch: par_t[ch][:, b:b + 1]                 # noqa: E731
                ddc = lambda ch: par_t[ch][:, BL + b:BL + b + 1]       # noqa: E731
                mags = mags_of[b]

                # p1[p, ch*T+t] = sp[ch*128+p] * (t+1): one GpSimd op with
                # tb broadcast over chunks and spB broadcast over t
                p1 = tfpool.tile([128, FD], F32, tag="tf")
                v3f = lambda tl: tl[:].rearrange(                     # noqa: E731
                    "p (c t) -> p c t", c=NCH)
                tb_b = tb_t[:].rearrange("p (o t) -> p o t", o=1).to_broadcast(
                    (128, NCH, T))
                spb_b = spb_t[:].rearrange(
                    "p (c bb) -> p c bb", bb=BL)[:, :, b:b + 1].to_broadcast(
                    (128, NCH, T))
                nc.vector.tensor_tensor(
                    v3f(p1), tb_b, spb_b, mybir.AluOpType.mult
                )
                # acc = dd*Sn + p1: two full-width DVE ops
                ddb_b = ddb_t[:].rearrange(
                    "p (c bb) -> p c bb", bb=BL)[:, :, b:b + 1].to_broadcast(
                    (128, NCH, T))
                snd = tfpool.tile([128, FD], F32, tag="tf")
                nc.vector.tensor_tensor(
                    v3f(snd), v3f(sn_t_), ddb_b, mybir.AluOpType.mult
                )
                acc = tfpool.tile([128, FD], F32, tag="tf")
                nc.vector.tensor_tensor(
                    acc[:], snd[:], p1[:], mybir.AluOpType.add
                )

                red = tfpool.tile([128, FD], F32, tag="tf")
                nc.vector._custom_dve(
                    _RR_OP, out=red[:], in0=acc[:],
                    s0=INV2PI, s1=MAGIC, imm2=TWO_PI_F32,
                )
                redc = tfpool.tile([128, FD], F32, tag="tf")
                nc.vector.add_range_wrap(redc[:], red[:], PI / 2, PI, TWO_PI)

                sinv = thpool.tile([128, FD], BF16, tag="th")
                nc.scalar.activation(sinv[:], red[:], mybir.ActivationFunctionType.Sin)
                cosv = thpool.tile([128, FD], BF16, tag="th")
                nc.scalar.activation(cosv[:], redc[:], mybir.ActivationFunctionType.Sin)

                re_t = spool.tile([128, FD], BF16, tag="re")
                im_t = spool.tile([128, FD], BF16, tag="im")
                # contiguous bf16 products (DVE 2x mode)
                nc.vector.tensor_tensor(
                    re_t[:], mags[:], cosv[:], mybir.AluOpType.mult
                )
                nc.vector.tensor_tensor(
                    im_t[:], mags[:], sinv[:], mybir.AluOpType.mult
                )

                # sigma-combine: rea[c,t] = sgn*re[c,t-1] + re[c,t].
                # One full-width shifted STT; the t=0 column of each chunk
                # picks up the previous chunk's t=127 value, so overwrite
                # those 9 columns with a strided copy afterwards (Pool).
                rea = spool.tile([128, FD], BF16, tag="rea")
                ima = spool.tile([128, FD], BF16, tag="ima")
                nc.vector.scalar_tensor_tensor(
                    rea[:, 1:], re_t[:, :FD - 1], sg_t[:], re_t[:, 1:],
                    mybir.AluOpType.mult, mybir.AluOpType.add,
                )
                nc.vector.scalar_tensor_tensor(
                    ima[:, 1:], im_t[:, :FD - 1], sg_t[:], im_t[:, 1:],
                    mybir.AluOpType.mult, mybir.AluOpType.add,
                )
                nc.gpsimd.tensor_copy(v3f(rea)[:, :, 0:1], v3f(re_t)[:, :, 0:1])
                nc.gpsimd.tensor_copy(v3f(ima)[:, :, 0:1], v3f(im_t)[:, :, 0:1])

                ps = ppool.tile([128, HOP], F32, tag="ps")
                pairs = []
                for ch in range(NCH):
                    pairs.append((rea, cm_t[ch], ch))
                    if ch < NCH - 1:
                        pairs.append((ima, sm_t[ch], ch))
                n_mm = len(pairs)
                for j in range(2):
                    for idx, (spec_t, mat_t, ch) in enumerate(pairs):
                        nc.tensor.matmul(
                            ps[:, j * 512:(j + 1) * 512],
                            spec_t[:, ch * T:(ch + 1) * T],
                            mat_t[:, j * 512:(j + 1) * 512],
                            start=(idx == 0), stop=(idx == n_mm - 1),
                        )

                ola = opool.tile([128, HOP], F32, tag="ola")
                nc.scalar.copy(ola[:], ps[:])
                nc.sync.dma_start(
                    out=out_d[b].rearrange("(t r) -> t r", t=T), in_=ola[:]
                )

    nc.compile()
    _CACHE["nc"] = nc
    return nc


def _prep_inputs(amp, phase, decay, phase_dither):
    """Host prep: flatten, derive per-(b,c) scalars, pad, pack to (CP, 4B)."""

    def flat(x):
        return np.transpose(np.asarray(x, np.float32), (0, 2, 1)).reshape(B, C)

    amp_f, phase_f, decay_f, dith_f = map(flat, (amp, phase, decay, phase_dither))
    gg = (BASE_RES + (1.0 / (1.0 + np.exp(-decay_f))) * (1.0 - BASE_RES) * RES_FACTOR)
    lc = np.log(gg)
    lnsm = 2.0 * np.log(np.maximum(amp_f, 1e-30))
    sp = np.tanh(phase_f) * np.pi
    dd = np.tanh(dith_f)

    def padT(x, padval):
        o = np.full((CP, B), padval, dtype=np.float32)
        o[:C, :] = x.T.astype(np.float32)
        return o

    return padT(sp, 1.0), padT(dd, 0.5), padT(lc, -0.2), padT(lnsm, -1.4)


def kernel(amp, phase, decay, phase_dither):
    sn_h, cm, smat, tb, sgn = _constants()
    sp, dd, gg, sm = _prep_inputs(amp, phase, decay, phase_dither)
    nc = _build_program()

    def chunk_major(x):  # (CP, BL) -> (128, NCH*BL): [p, ch*BL+b]
        return np.ascontiguousarray(
            x.reshape(NCH, 128, BL).transpose(1, 0, 2).reshape(128, NCH * BL)
        )

    in_maps = []
    for core in range(N_CORES):
        bs = slice(core * BL, (core + 1) * BL)
        par = np.concatenate([sp[:, bs], dd[:, bs], gg[:, bs], sm[:, bs]], axis=1)
        in_maps.append({
            "par": np.ascontiguousarray(par),
            "spB": chunk_major(sp[:, bs]),
            "ddB": chunk_major(dd[:, bs]),
            "snt": sn_h[bs],
            "tb": tb,
            "sgn": sgn,
            "cmat": cm,
            "smat": smat,
        })

    res = bass_utils.run_bass_kernel_spmd(
        nc, in_maps, core_ids=list(range(N_CORES))
    )
    out = np.concatenate([r["out"] for r in res.results], axis=0)  # (256, 131072)
    return out.reshape(1, 1, N_RES, EXPR, N_SAMPLES)


# revision 26
# speedup vs baseline: 1.5052x; 1.1486x over previous
"""Trainium2 Bass kernel for FFTResonanceBlock.

Math (per flattened resonator b of B=256, freq bin c of 1025, frame t of 128):
  coeffs = 0.5 + sigmoid(decay)*0.5*0.99
  mags[t]  = amp^2 * coeffs^(t+1)              (geometric scan on DVE)
  phase[t] = (t+1)*tanh(phase)*pi + tanh(dith)*Sn[t]   (Sn = const noise cumsum)
  spec = mags * exp(i*phase); frames = irfft(spec, 2048); overlap-add hop 1024.

Device strategy (8 cores, 32 resonators each, embarrassingly parallel):
  - tiles (c_chunk=128 partitions, t=128 free) per resonator
  - sin/cos via magic-number round + cody-waite reduction + Sin LUT
  - irfft + overlap-add as ONE halved matmul using
      D[c, 1024+r] = (-1)^c D[c, r]:
    out[t,r] = sum_c (re[c,t] + (-1)^c re[c,t-1]) C[c,r]
             + (im[c,t] + (-1)^c im[c,t-1]) S[c,r]
    with the sigma-combine done on GpSimd, products split DVE/GpSimd.
"""
import sys

sys.path.insert(0, "/opt/trn_rl_repo")

import numpy as np
import ml_dtypes

import concourse.bass as bass  # noqa: F401
import concourse.mybir as mybir
import concourse.tile as tile
from concourse import bacc, bass_utils

# ---- problem constants (hardcoded per spec) ----
N_CORES = 8
N_RES, EXPR = 64, 4
B = N_RES * EXPR          # 256 flattened resonators
BL = B // N_CORES         # 32 per core
C = 1025                  # rfft bins for window 2048
CP = 1152                 # padded to 9*128
NCH = CP // 128           # 9 c-chunks
T = 128                   # frames
W = 2048                  # window
HOP = 1024
N_SAMPLES = T * HOP       # 131072
BASE_RES = 0.5
RES_FACTOR = 0.99

PI = float(np.pi)
TWO_PI = 2.0 * np.pi
INV2PI = float(1.0 / TWO_PI)
MAGIC = float(np.float32(1.5 * 2**23))
CW1 = float(np.float32(6.28125))
CW2 = float(np.float32(TWO_PI - 6.28125))
CW3 = float(np.float32(TWO_PI - 6.28125 - float(np.float32(TWO_PI - 6.28125))))

F32 = mybir.dt.float32
BF16 = mybir.dt.bfloat16
TWO_PI_F32 = float(np.float32(TWO_PI))

_CACHE: dict = {}


def _register_range_reduce_op():
    """Custom DVE op: red = x - round(x/2pi)*fl(2pi) in ONE instruction
    (magic-number round + single-constant reduction, ~2.7e-5 rad error)."""
    from concourse import dve_ops as dvo
    from concourse.dve_spec import Spec, Src0, C0, C1, C2

    name = "RANGE_REDUCE_2PI_ANT"
    if name in dvo._SUB_OPCODE_FOR_NAME:
        return next(o for o in dvo.OPS if o.name == name)
    k = (Src0 * C0 + C1) - C1
    op = dvo.DveOp(
        name,
        Spec(
            body=Src0 - k * C2,
            reference=lambda in0, s0, s1, imm2: (
                in0.astype(np.float32)
                - ((in0.astype(np.float32) * np.float32(s0) + np.float32(s1))
                   - np.float32(s1)) * np.float32(imm2)
            ),
        ),
        subdim=False,
        uops_sha={"v3": "7f89aefee880b3b9", "v4": "7f89aefee880b3b9"},
    )
    dvo.OPS.append(op)
    dvo._SUB_OPCODE_FOR_NAME[name] = max(dvo._SUB_OPCODE_FOR_NAME.values()) + 1
    dvo.CUSTOM_DVE_SPECS[name] = op.spec
    return op


def _register_sum_range_reduce_op():
    """Custom DVE op: red = (a+b) - round((a+b)/2pi)*fl(2pi) — fuses the
    phase-accumulation add with the range reduction."""
    from concourse import dve_ops as dvo
    from concourse.dve_spec import Spec, Src0, Src1, C0, C1, C2

    name = "SUM_RANGE_REDUCE_2PI_ANT"
    if name in dvo._SUB_OPCODE_FOR_NAME:
        return next(o for o in dvo.OPS if o.name == name)
    X = Src0 + Src1
    k = (X * C0 + C1) - C1
    op = dvo.DveOp(
        name,
        Spec(
            body=X - k * C2,
            reference=lambda in0, in1, s0, s1, imm2: (
                lambda Xv: Xv - ((Xv * np.float32(s0) + np.float32(s1))
                                 - np.float32(s1)) * np.float32(imm2)
            )(in0.astype(np.float32) + in1.astype(np.float32)),
        ),
        subdim=False,
        uops_sha={"v3": "e093ffe4004aa4fb", "v4": "e093ffe4004aa4fb"},
    )
    dvo.OPS.append(op)
    dvo._SUB_OPCODE_FOR_NAME[name] = max(dvo._SUB_OPCODE_FOR_NAME.values()) + 1
    dvo.CUSTOM_DVE_SPECS[name] = op.spec
    return op


_RR_OP = _register_range_reduce_op()
_RR2_OP = _register_sum_range_reduce_op()


def _constants():
    """Input-independent constants: noise cumsum, halved DFT mats, t-vec, sign."""
    if "consts" in _CACHE:
        return _CACHE["consts"]
    import jax

    cpu = jax.devices("cpu")[0]
    with jax.default_device(cpu):
        noise = jax.random.uniform(
            jax.random.key(42), (B, T, C), minval=-1.0, maxval=1.0
        )
        noise = np.asarray(noise, dtype=np.float32)
    sn = np.cumsum(noise, axis=1, dtype=np.float32)        # (B, T, C)
    sn_p = np.zeros((B, CP, T), dtype=np.float32)
    sn_p[:, :C, :] = np.transpose(sn, (0, 2, 1))
    # descriptor-friendly layout: (B, p, ch, t) so each partition is one
    # contiguous 4.6KB run
    sn_h = np.ascontiguousarray(
        sn_p.reshape(B, NCH, 128, T).transpose(0, 2, 1, 3)
    ).reshape(B, 128, NCH * T)

    # halved DFT matrices, cols 0..1023 (second half folded via (-1)^c)
    k = np.arange(CP, dtype=np.float64)[:, None]
    n = np.arange(HOP, dtype=np.float64)[None, :]
    ang = 2.0 * np.pi * k * n / W
    w = np.full((CP, 1), 2.0 / W)
    w[0, 0] = 1.0 / W
    w[C - 1, 0] = 1.0 / W
    w[C:, 0] = 0.0
    cm = (w * np.cos(ang)).astype(ml_dtypes.bfloat16)              # (1152, 1024)
    smat = (-(w * np.sin(ang))[:1024]).astype(ml_dtypes.bfloat16)  # (1024, 1024)

    tb = np.broadcast_to(
        np.arange(1, T + 1, dtype=np.float32)[None, :], (128, T)
    ).copy()
    sgn = np.where(np.arange(128) % 2 == 0, 1.0, -1.0).astype(np.float32)
    sgn = sgn.reshape(128, 1)

    _CACHE["consts"] = (sn_h, cm, smat, tb, sgn)
    return _CACHE["consts"]


def _build_program():
    if "nc" in _CACHE:
        return _CACHE["nc"]

    nc = bacc.Bacc("TRN2", target_bir_lowering=False, debug=False, num_devices=1)

    par_d = nc.dram_tensor("par", (CP, 4 * BL), F32, kind="ExternalInput").ap()
    spb_d = nc.dram_tensor("spB", (128, NCH * BL), F32, kind="ExternalInput").ap()
    ddb_d = nc.dram_tensor("ddB", (128, NCH * BL), F32, kind="ExternalInput").ap()
    sn_d = nc.dram_tensor("snt", (BL, 128, NCH * T), F32, kind="ExternalInput").ap()
    tb_d = nc.dram_tensor("tb", (128, T), F32, kind="ExternalInput").ap()
    sg_d = nc.dram_tensor("sgn", (128, 1), F32, kind="ExternalInput").ap()
    cm_d = nc.dram_tensor("cmat", (CP, HOP), BF16, kind="ExternalInput").ap()
    sm_d = nc.dram_tensor("smat", (1024, HOP), BF16, kind="ExternalInput").ap()
    out_d = nc.dram_tensor("out", (BL, N_SAMPLES), F32, kind="ExternalOutput").ap()

    with tile.TileContext(nc) as tc:
        with (
            tc.tile_pool(name="const", bufs=1) as cpool,
            tc.tile_pool(name="sn", bufs=3) as snpool,
            tc.tile_pool(name="tf", bufs=12) as tfpool,
            tc.tile_pool(name="th", bufs=5) as thpool,
            tc.tile_pool(name="mg", bufs=10) as mgpool,
            tc.tile_pool(name="spec", bufs=2) as spool,
            tc.tile_pool(name="ola", bufs=3) as opool,
            tc.tile_pool(name="ps", bufs=4, space="PSUM") as ppool,
        ):
            # small constants first (unblocks the b=0 compute chain)
            par_t = []
            for ch in range(NCH):
                t_ = cpool.tile([128, 4 * BL], F32, tag=f"par{ch}")
                nc.sync.dma_start(out=t_[:], in_=par_d[ch * 128:(ch + 1) * 128, :])
                par_t.append(t_)
            tb_t = cpool.tile([128, T], F32, tag="tb")
            nc.sync.dma_start(out=tb_t[:], in_=tb_d[:])
            sg_t = cpool.tile([128, 1], F32, tag="sgn")
            nc.sync.dma_start(out=sg_t[:], in_=sg_d[:])
            spb_t = cpool.tile([128, NCH * BL], F32, tag="spB")
            nc.sync.dma_start(out=spb_t[:], in_=spb_d[:])
            ddb_t = cpool.tile([128, NCH * BL], F32, tag="ddB")
            nc.sync.dma_start(out=ddb_t[:], in_=ddb_d[:])
            # DFT matrices on the SWDGE queue, chunk 0 first
            cm_t, sm_t = [], []
            for ch in range(NCH):
                ct = cpool.tile([128, HOP], BF16, tag=f"cm{ch}")
                nc.gpsimd.dma_start(out=ct[:], in_=cm_d[ch * 128:(ch + 1) * 128, :])
                cm_t.append(ct)
                if ch < NCH - 1:
                    st = cpool.tile([128, HOP], BF16, tag=f"sm{ch}")
                    nc.gpsimd.dma_start(
                        out=st[:], in_=sm_d[ch * 128:(ch + 1) * 128, :]
                    )
                    sm_t.append(st)

            FD = NCH * T      # 1152
            FDP = NCH * (T + 1)  # 1161, padded spec layout
            GRP = 8           # resonators per exp/sin table-load phase

            for g in range(BL // GRP):
              mags_of = {}
              # ---- phase A: decay envelopes (Exp table set) ----
              # mags = exp(lc*(t+1) + lnsm), fused into the activation's
              # per-partition scale/bias — no DVE involvement.
              for b in range(g * GRP, (g + 1) * GRP):
                mg = mgpool.tile([128, FD], BF16, tag="mg")
                for ch in range(NCH):
                    sl = slice(ch * T, (ch + 1) * T)
                    nc.scalar.activation(
                        mg[:, sl], tb_t[:], mybir.ActivationFunctionType.Exp,
                        bias=par_t[ch][:, 3 * BL + b:3 * BL + b + 1],
                        scale=par_t[ch][:, 2 * BL + b:2 * BL + b + 1],
                    )
                mags_of[b] = mg
              # ---- phase B: phase accumulation + sin/cos + DFT (Sin set) ----
              for b in range(g * GRP, (g + 1) * GRP):
                sn_t_ = snpool.tile([128, FD], F32, tag="sn")
                nc.sync.dma_start(out=sn_t_[:], in_=sn_d[b])

                spc = lambda ch: par_t[ch][:, b:b + 1]                 # noqa: E731
                ddc = lambda ch: par_t[ch][:, BL + b:BL + b + 1]       # noqa: E731
                mags = mags_of[b]

                # p1[p, ch*T+t] = sp[ch*128+p] * (t+1): one GpSimd op with
                # tb broadcast over chunks and spB broadcast over t
                p1 = tfpool.tile([128, FD], F32, tag="tf")
                v3f = lambda tl: tl[:].rearrange(                     # noqa: E731
                    "p (c t) -> p c t", c=NCH)
                tb_b = tb_t[:].rearrange("p (o t) -> p o t", o=1).to_broadcast(
                    (128, NCH, T))
                spb_b = spb_t[:].rearrange(
                    "p (c bb) -> p c bb", bb=BL)[:, :, b:b + 1].to_broadcast(
                    (128, NCH, T))
                nc.vector.tensor_tensor(
                    v3f(p1), tb_b, spb_b, mybir.AluOpType.mult
                )
                # acc = dd*Sn + p1: two full-width DVE ops
                ddb_b = ddb_t[:].rearrange(
                    "p (c bb) -> p c bb", bb=BL)[:, :, b:b + 1].to_broadcast(
                    (128, NCH, T))
                snd = tfpool.tile([128, FD], F32, tag="tf")
                nc.vector.tensor_tensor(
                    v3f(snd), v3f(sn_t_), ddb_b, mybir.AluOpType.mult
                )
                # red = reduce(snd + p1): fused add + magic-round + 2pi reduce
                red = tfpool.tile([128, FD], F32, tag="tf")
                nc.vector._custom_dve(
                    _RR2_OP, out=red[:], in0=snd[:], in1=p1[:],
                    s0=INV2PI, s1=MAGIC, imm2=TWO_PI_F32,
                )
                redc = tfpool.tile([128, FD], F32, tag="tf")
                nc.vector.add_range_wrap(redc[:], red[:], PI / 2, PI, TWO_PI)

                sinv = thpool.tile([128, FD], BF16, tag="th")
                nc.scalar.activation(sinv[:], red[:], mybir.ActivationFunctionType.Sin)
                cosv = thpool.tile([128, FD], BF16, tag="th")
                nc.scalar.activation(cosv[:], redc[:], mybir.ActivationFunctionType.Sin)

                re_t = spool.tile([128, FD], BF16, tag="re")
                im_t = spool.tile([128, FD], BF16, tag="im")
                # contiguous bf16 products (DVE 2x mode)
                nc.vector.tensor_tensor(
                    re_t[:], mags[:], cosv[:], mybir.AluOpType.mult
                )
                nc.vector.tensor_tensor(
                    im_t[:], mags[:], sinv[:], mybir.AluOpType.mult
                )

                # sigma-combine: rea[c,t] = sgn*re[c,t-1] + re[c,t].
                # One full-width shifted STT; the t=0 column of each chunk
                # picks up the previous chunk's t=127 value, so overwrite
                # those 9 columns with a strided copy afterwards (Pool).
                rea = spool.tile([128, FD], BF16, tag="rea")
                ima = spool.tile([128, FD], BF16, tag="ima")
                nc.vector.scalar_tensor_tensor(
                    rea[:, 1:], re_t[:, :FD - 1], sg_t[:], re_t[:, 1:],
                    mybir.AluOpType.mult, mybir.AluOpType.add,
                )
                nc.vector.scalar_tensor_tensor(
                    ima[:, 1:], im_t[:, :FD - 1], sg_t[:], im_t[:, 1:],
                    mybir.AluOpType.mult, mybir.AluOpType.add,
                )
                nc.gpsimd.tensor_copy(v3f(rea)[:, :, 0:1], v3f(re_t)[:, :, 0:1])
                nc.gpsimd.tensor_copy(v3f(ima)[:, :, 0:1], v3f(im_t)[:, :, 0:1])

                ps = ppool.tile([128, HOP], F32, tag="ps")
                pairs = []
                for ch in range(NCH):
                    pairs.append((rea, cm_t[ch], ch))
                    if ch < NCH - 1:
                        pairs.append((ima, sm_t[ch], ch))
                n_mm = len(pairs)
                for j in range(2):
                    for idx, (spec_t, mat_t, ch) in enumerate(pairs):
                        nc.tensor.matmul(
                            ps[:, j * 512:(j + 1) * 512],
                            spec_t[:, ch * T:(ch + 1) * T],
                            mat_t[:, j * 512:(j + 1) * 512],
                            start=(idx == 0), stop=(idx == n_mm - 1),
                        )

                ola = opool.tile([128, HOP], F32, tag="ola")
                nc.scalar.copy(ola[:], ps[:])
                nc.sync.dma_start(
                    out=out_d[b].rearrange("(t r) -> t r", t=T), in_=ola[:]
                )

    nc.compile()
    _CACHE["nc"] = nc
    return nc


def _prep_inputs(amp, phase, decay, phase_dither):
    """Host prep: flatten, derive per-(b,c) scalars, pad, pack to (CP, 4B)."""

    def flat(x):
        return np.transpose(np.asarray(x, np.float32), (0, 2, 1)).reshape(B, C)

    amp_f, phase_f, decay_f, dith_f = map(flat, (amp, phase, decay, phase_dither))
    gg = (BASE_RES + (1.0 / (1.0 + np.exp(-decay_f))) * (1.0 - BASE_RES) * RES_FACTOR)
    lc = np.log(gg)
    lnsm = 2.0 * np.log(np.maximum(amp_f, 1e-30))
    sp = np.tanh(phase_f) * np.pi
    dd = np.tanh(dith_f)

    def padT(x, padval):
        o = np.full((CP, B), padval, dtype=np.float32)
        o[:C, :] = x.T.astype(np.float32)
        return o

    return padT(sp, 1.0), padT(dd, 0.5), padT(lc, -0.2), padT(lnsm, -1.4)


def kernel(amp, phase, decay, phase_dither):
    sn_h, cm, smat, tb, sgn = _constants()
    sp, dd, gg, sm = _prep_inputs(amp, phase, decay, phase_dither)
    nc = _build_program()

    def chunk_major(x):  # (CP, BL) -> (128, NCH*BL): [p, ch*BL+b]
        return np.ascontiguousarray(
            x.reshape(NCH, 128, BL).transpose(1, 0, 2).reshape(128, NCH * BL)
        )

    in_maps = []
    for core in range(N_CORES):
        bs = slice(core * BL, (core + 1) * BL)
        par = np.concatenate([sp[:, bs], dd[:, bs], gg[:, bs], sm[:, bs]], axis=1)
        in_maps.append({
            "par": np.ascontiguousarray(par),
            "spB": chunk_major(sp[:, bs]),
            "ddB": chunk_major(dd[:, bs]),
            "snt": sn_h[bs],
            "tb": tb,
            "sgn": sgn,
            "cmat": cm,
            "smat": smat,
        })

    res = bass_utils.run_bass_kernel_spmd(
        nc, in_maps, core_ids=list(range(N_CORES))
    )
    out = np.concatenate([r["out"] for r in res.results], axis=0)  # (256, 131072)
    return out.reshape(1, 1, N_RES, EXPR, N_SAMPLES)
